# revision 1
# baseline (speedup 1.0000x reference)
"""DIN attention layer kernel for Trainium2 (8 NeuronCores, data-parallel over batch).

Reference computation (per batch b):
    att = [q, k, q-k, q*k]            # [T, 4M]
    h1  = relu(att @ W1 + b1)         # [T, D]
    h2  = relu(h1 @ W2 + b2)          # [T, D]
    s   = h2 @ w_score + b_score      # [T, 1]
    attn = softmax(s.T + mask * -1e9) # [1, T]
    out = attn @ values               # [1, D]

Key optimizations:
  * Data-parallel: 8 batches per core (B=64 over 8 cores).
  * Algebraic reassociation of the concat matmul:
        att @ W1 = q@(W1a+W1c) + k@(W1b-W1c) + (q*k)@W1d
    The q term is a per-batch row vector folded into the layer-1 bias,
    so the effective contraction is 512 instead of 1024 (mm1 halved).
  * Transposed-activation layout [feature, token]: weights W1/W2 are used
    as-stored for lhsT (no weight transposes); only keys need a transpose,
    done on the PE via identity matmul.
  * bf16 matmuls (fp32 PSUM accumulation); bias/softmax kept in fp32.
  * Softmax without max-subtraction (scores are O(1); masked lanes are
    exp(-1e9) = 0), sum fused into the Exp activation via accum_out.
  * Software-pipelined emission: attn@values for batch b is emitted inside
    batch b+1's block so the PE never waits on the softmax chain.
"""

import numpy as np

P = 128
B = 8          # batches per core
T = 1024       # tokens
M = 256        # key feature dim
D = 1024       # hidden dim
MC = M // P    # key-feature chunks (2)
DC = D // P    # hidden chunks (8)
TC = T // P    # token chunks (8)
NH = 2         # free-dim halves of 512
NEG = -1.0e9

_built = None


def _ns(h):
    return slice(h * 512, (h + 1) * 512)


def _build():
    import os
    stage = int(os.environ.get("DIN_STAGE", "5"))
    setup_n = int(os.environ.get("DIN_SETUP_N", "99"))
    import concourse.bass as bass
    import concourse.bacc as bacc
    import concourse.mybir as mybir
    import concourse.tile as tile
    from concourse.masks import make_identity
    from contextlib import ExitStack

    F32 = mybir.dt.float32
    BF16 = mybir.dt.bfloat16
    AF = mybir.ActivationFunctionType
    OP = mybir.AluOpType

    nc = bacc.Bacc("TRN2")
    q_d = nc.dram_tensor("query", [B, M], F32, kind="ExternalInput").ap()
    k_d = nc.dram_tensor("keys", [B, T, M], F32, kind="ExternalInput").ap()
    v_d = nc.dram_tensor("values", [B, T, D], F32, kind="ExternalInput").ap()
    m_d = nc.dram_tensor("mask", [B, T], F32, kind="ExternalInput").ap()
    w1_d = nc.dram_tensor("W1", [4 * M, D], F32, kind="ExternalInput").ap()
    b1_d = nc.dram_tensor("b1", [D], F32, kind="ExternalInput").ap()
    w2_d = nc.dram_tensor("W2", [D, D], F32, kind="ExternalInput").ap()
    b2_d = nc.dram_tensor("b2", [D], F32, kind="ExternalInput").ap()
    ws_d = nc.dram_tensor("w_score", [D, 1], F32, kind="ExternalInput").ap()
    out_d = nc.dram_tensor("out", [B, D], F32, kind="ExternalOutput").ap()

    with tile.TileContext(nc) as tc, ExitStack() as ctx:
        cons = ctx.enter_context(tc.tile_pool(name="cons", bufs=1))
        kraw = ctx.enter_context(tc.tile_pool(name="kraw", bufs=2))
        xpool = ctx.enter_context(tc.tile_pool(name="xpool", bufs=2))
        h1pool = ctx.enter_context(tc.tile_pool(name="h1p", bufs=1))
        h2pool = ctx.enter_context(tc.tile_pool(name="h2p", bufs=2))
        vpool = ctx.enter_context(tc.tile_pool(name="vp", bufs=1))
        small = ctx.enter_context(tc.tile_pool(name="small", bufs=2))
        dram = ctx.enter_context(tc.tile_pool(name="dram", bufs=2, space="DRAM"))
        psum_mm = ctx.enter_context(tc.tile_pool(name="psmm", bufs=4, space="PSUM"))
        psum_sc = ctx.enter_context(tc.tile_pool(name="pssc", bufs=2, space="PSUM"))
        psum_vec = ctx.enter_context(tc.tile_pool(name="psvec", bufs=2, space="PSUM"))

        # ---- one-time setup -------------------------------------------------
        identity = cons.tile([P, P], F32)
        make_identity(nc, identity)

        # striped per-channel vectors: [p, c] = vec[c*128 + p]
        b2_sb = cons.tile([P, DC], F32)
        ws_f = cons.tile([P, DC], F32)
        ws_sb = cons.tile([P, DC], BF16)
        qt_f = cons.tile([P, MC, B], F32)
        qt_b = cons.tile([P, MC, B], BF16)
        if setup_n >= 2:
            nc.gpsimd.dma_start(b2_sb, b2_d.rearrange("(c p) -> p c", p=P))
            nc.gpsimd.dma_start(ws_f, ws_d.rearrange("(c p) one -> p (c one)", p=P))
            nc.vector.tensor_copy(ws_sb, ws_f)
            for c in range(MC):
                nc.gpsimd.dma_start(
                    qt_f[:, c, :], q_d[:, c * P:(c + 1) * P].rearrange("b p -> p b")
                )
            nc.vector.tensor_copy(qt_b, qt_f)
        else:
            nc.vector.memset(b2_sb, 0.0)
            nc.vector.memset(ws_sb, 0.01)
            nc.vector.memset(qt_f, 0.01)
            nc.vector.memset(qt_b, 0.01)

        # weights in bf16, contraction dim on partitions (as stored);
        # fp32 DMA staging through the shared kraw slots, cast on the DVE
        w2_sb = cons.tile([P, DC, D], BF16)
        if setup_n >= 3:
            for g in range(4):
                wtmp = kraw.tile([P, MC, D], F32, tag="kraw", name=f"w2tmp{g}")
                nc.sync.dma_start(
                    wtmp, w2_d[g * M:(g + 1) * M, :].rearrange("(c p) d -> p c d", p=P)
                )
                nc.vector.tensor_copy(w2_sb[:, 2 * g:2 * g + 2, :], wtmp)
        else:
            nc.vector.memset(w2_sb, 0.01)

        w1qc = cons.tile([P, MC, D], BF16)   # W1a + W1c
        w1bc = cons.tile([P, MC, D], BF16)   # W1b - W1c
        w1d_sb = cons.tile([P, MC, D], BF16)  # W1d
        if setup_n >= 4:
            tmp_a = kraw.tile([P, MC, D], F32, tag="kraw")
            tmp_c = kraw.tile([P, MC, D], F32, tag="kraw")
            nc.sync.dma_start(tmp_a, w1_d[0:M, :].rearrange("(c p) d -> p c d", p=P))
            nc.sync.dma_start(tmp_c, w1_d[2 * M:3 * M, :].rearrange("(c p) d -> p c d", p=P))
            nc.vector.tensor_add(w1qc, tmp_a, tmp_c)
            tmp_b = kraw.tile([P, MC, D], F32, tag="kraw")
            nc.sync.dma_start(tmp_b, w1_d[M:2 * M, :].rearrange("(c p) d -> p c d", p=P))
            nc.vector.tensor_sub(w1bc, tmp_b, tmp_c)
            tmp_d = kraw.tile([P, MC, D], F32, tag="kraw")
            nc.sync.dma_start(tmp_d, w1_d[3 * M:4 * M, :].rearrange("(c p) d -> p c d", p=P))
            nc.vector.tensor_copy(w1d_sb, tmp_d)
        else:
            nc.vector.memset(w1qc, 0.01)
            nc.vector.memset(w1bc, 0.01)
            nc.vector.memset(w1d_sb, 0.01)

        # RT[p, b, j] = R^T[j*128+p, b] + b1[j*128+p], computed directly on
        # the PE (out = w1qc_chunk.T @ qt_chunk gives [d, b]) so no DMA
        # transpose/scatter is ever needed; bias added during the psum drain
        b1_sb = cons.tile([P, DC], F32)
        nc.gpsimd.dma_start(b1_sb, b1_d.rearrange("(c p) -> p c", p=P))
        rt = cons.tile([P, B, DC], F32)
        if setup_n >= 5:
            for j in range(DC):
                rt_ps = psum_vec.tile([P, B], F32, tag="vec", name=f"rt_ps{j}")
                for c in range(MC):
                    nc.tensor.matmul(
                        rt_ps, w1qc[:, c, j * P:(j + 1) * P], qt_b[:, c, :],
                        start=(c == 0), stop=(c == MC - 1),
                    )
                nc.vector.tensor_scalar(
                    rt[:, :, j], rt_ps, b1_sb[:, j:j + 1], None,
                    op0=OP.add,
                )
        else:
            nc.vector.memset(rt, 0.0)

        if stage == 0:
            o1 = small.tile([1, D], F32, tag="osb")
            nc.vector.tensor_copy(o1[:, 0:DC], rt[0:1, 0, :])
            nc.vector.memset(o1[:, DC:], 0.0)
            for b in range(B):
                nc.sync.dma_start(out_d[b:b + 1, :], o1)
            _stage0 = True
        else:
            _stage0 = False

        # ---- per-batch pipeline --------------------------------------------
        # state carried from batch b to block b+1 (deferred attn @ values)
        carry = {}

        def emit_attn_values(b):
            st = carry.pop(b)
            out_ps = [psum_vec.tile([1, 512], F32, tag="vec", name=f"o_ps{h}") for h in range(NH)]
            for h in range(NH):
                for c in range(TC):
                    nc.tensor.matmul(
                        out_ps[h],
                        st["attn_t"][:, c:c + 1],
                        st["vals"][:, c, _ns(h)],
                        start=(c == 0), stop=(c == TC - 1),
                    )
            out_sb = small.tile([1, D], F32, tag="osb")
            for h in range(NH):
                nc.vector.tensor_scalar_mul(out_sb[:, _ns(h)], out_ps[h], st["rec"])
            nc.sync.dma_start(out_d[b:b + 1, :], out_sb)

        for b in range(B if not _stage0 else 0):
            # load keys and transpose on the PE: X[p, c, t] = keys[b, t, c*128+p]
            keys_b = kraw.tile([P, TC, M], F32, tag="kraw")
            nc.sync.dma_start(keys_b, k_d[b].rearrange("(to p) m -> p to m", p=P))
            mask_t = small.tile([1, T], F32, tag="mask")
            nc.sync.dma_start(mask_t, m_d[b:b + 1, :])

            x_t = xpool.tile([P, MC, T], BF16, tag="X")
            for to in range(TC):
                for c in range(MC):
                    tp = psum_mm.tile([P, P], F32, tag="mm")
                    nc.tensor.transpose(tp, keys_b[:, to, c * P:(c + 1) * P], identity)
                    nc.vector.tensor_copy(x_t[:, c, to * P:(to + 1) * P], tp)

            if stage == 10:
                o1 = small.tile([1, D], F32, tag="osb")
                nc.vector.tensor_copy(o1[:, 0:D // 2], x_t[0:1, 0, 0:D].bitcast(F32))
                nc.vector.memset(o1[:, D // 2:], 0.0)
                nc.sync.dma_start(out_d[b:b + 1, :], o1)
                carry[b] = None
                continue

            # deferred attn@values for the previous batch sits here so the PE
            # is never blocked on the (latency-bound) softmax chain
            if b > 0 and stage >= 5:
                emit_attn_values(b - 1)

            vals_f = vpool.tile([P, TC, D], F32, tag="vals_f")
            nc.sync.dma_start(vals_f, v_d[b].rearrange("(to p) d -> p to d", p=P))
            vals = vpool.tile([P, TC, D], BF16, tag="vals")
            nc.vector.tensor_copy(vals, vals_f)

            # QK[p, c, t] = q[b, c*128+p] * X[p, c, t]
            qk = xpool.tile([P, MC, T], BF16, tag="QK")
            for c in range(MC):
                nc.vector.tensor_scalar_mul(qk[:, c, :], x_t[:, c, :], qt_f[:, c, b:b + 1])

            if stage == 1:
                o1 = small.tile([1, D], F32, tag="osb")
                nc.vector.tensor_copy(o1[:, 0:D // 2], x_t[0:1, 0, 0:D].bitcast(F32))
                nc.vector.memset(o1[:, D // 2:], 0.0)
                nc.sync.dma_start(out_d[b:b + 1, :], o1)
                carry[b] = None
                continue

            # mm1: H1[d, t] = relu(W1bc.T @ X + W1d.T @ QK + bias1)
            h1 = h1pool.tile([P, DC, T], BF16, tag="H1")
            for j in range(DC):
                for h in range(NH):
                    ps = psum_mm.tile([P, 512], F32, tag="mm")
                    for c in range(MC):
                        nc.tensor.matmul(
                            ps, w1bc[:, c, j * P:(j + 1) * P], x_t[:, c, _ns(h)],
                            start=(c == 0), stop=False,
                        )
                    for c in range(MC):
                        nc.tensor.matmul(
                            ps, w1d_sb[:, c, j * P:(j + 1) * P], qk[:, c, _ns(h)],
                            start=False, stop=(c == MC - 1),
                        )
                    nc.scalar.activation(
                        h1[:, j, _ns(h)], ps, AF.Relu, bias=rt[:, b, j:j + 1], scale=1.0
                    )

            if stage == 2:
                o1 = small.tile([1, D], F32, tag="osb")
                nc.vector.tensor_copy(o1[:, 0:D // 2], h1[0:1, 0, 0:D].bitcast(F32))
                nc.vector.memset(o1[:, D // 2:], 0.0)
                nc.sync.dma_start(out_d[b:b + 1, :], o1)
                carry[b] = None
                continue

            # mm2 + score: H2 chunks are consumed immediately by the score
            # matmuls (deferred by one j so the PE doesn't wait on the relu)
            score_ps = [psum_sc.tile([1, 512], F32, tag="sc", name=f"s_ps{h}") for h in range(NH)]
            h2_tiles = {}
            for j in range(DC):
                h2 = h2pool.tile([P, T], BF16, tag="H2")
                h2_tiles[j] = h2
                for h in range(NH):
                    ps = psum_mm.tile([P, 512], F32, tag="mm")
                    for c in range(DC):
                        nc.tensor.matmul(
                            ps, w2_sb[:, c, j * P:(j + 1) * P], h1[:, c, _ns(h)],
                            start=(c == 0), stop=(c == DC - 1),
                        )
                    nc.scalar.activation(
                        h2[:, _ns(h)], ps, AF.Relu, bias=b2_sb[:, j:j + 1], scale=1.0
                    )
                if j > 0:
                    jj = j - 1
                    h2_prev = h2_tiles.pop(jj)
                    for h in range(NH):
                        nc.tensor.matmul(
                            score_ps[h], ws_sb[:, jj:jj + 1], h2_prev[:, _ns(h)],
                            start=(jj == 0), stop=False, skip_group_check=True,
                        )
            jj = DC - 1
            h2_prev = h2_tiles.pop(jj)
            for h in range(NH):
                nc.tensor.matmul(
                    score_ps[h], ws_sb[:, jj:jj + 1], h2_prev[:, _ns(h)],
                    start=False, stop=True, skip_group_check=True,
                )

            if stage == 3:
                o1 = small.tile([1, D], F32, tag="osb")
                for h in range(NH):
                    nc.vector.tensor_copy(o1[:, _ns(h)], score_ps[h])
                nc.sync.dma_start(out_d[b:b + 1, :], o1)
                carry[b] = None
                continue

            # softmax (no max subtraction: scores are O(1), masked lanes
            # underflow to exactly 0). score = mask * -1e9 + raw_score
            score_sb = small.tile([1, T], F32, tag="ssb")
            for h in range(NH):
                nc.vector.scalar_tensor_tensor(
                    score_sb[:, _ns(h)], in0=mask_t[:, _ns(h)], scalar=NEG,
                    in1=score_ps[h], op0=OP.mult, op1=OP.add,
                )
            sum_sb = small.tile([1, 1], F32, tag="sum")
            exp_bf = small.tile([1, T], BF16, tag="expb")
            nc.scalar.activation(exp_bf, score_sb, AF.Exp, accum_out=sum_sb)
            rec = small.tile([1, 1], F32, tag="rec")
            nc.vector.reciprocal(rec, sum_sb)
            # attn_t[p, c] = exp_score[c*128 + p]  (partition-major for lhsT),
            # via a DRAM bounce to keep the SBUF write partition-outer
            attn_dram = dram.tile([1, T], BF16, tag="attn_dram")
            nc.sync.dma_start(attn_dram, exp_bf)
            attn_t = small.tile([P, TC], BF16, tag="attn")
            nc.sync.dma_start(
                attn_t, attn_dram.rearrange("one (c p) -> p (one c)", p=P)
            )
            if stage == 4:
                o1 = small.tile([1, D], F32, tag="osb")
                nc.vector.tensor_copy(o1[:, 0:TC], attn_t[0:1, :])
                nc.vector.memset(o1[:, TC:], 0.0)
                nc.sync.dma_start(out_d[b:b + 1, :], o1)
                carry[b] = None
                continue

            carry[b] = {"attn_t": attn_t, "vals": vals, "rec": rec}

        if stage >= 5 and not _stage0:
            emit_attn_values(B - 1)

    nc.compile()
    return nc


def _get_built():
    global _built
    if _built is None:
        _built = _build()
    return _built


N_CORES = 8


def make_in_maps(query, keys, values, mask, W1, b1, W2, b2, w_score, b_score=None):
    query = np.ascontiguousarray(np.asarray(query, dtype=np.float32).reshape(64, M))
    keys = np.ascontiguousarray(np.asarray(keys, dtype=np.float32))
    values = np.ascontiguousarray(np.asarray(values, dtype=np.float32))
    mask = np.ascontiguousarray(np.asarray(mask, dtype=np.float32).reshape(64, T))
    shared = {
        "W1": np.ascontiguousarray(np.asarray(W1, dtype=np.float32)),
        "b1": np.ascontiguousarray(np.asarray(b1, dtype=np.float32)),
        "W2": np.ascontiguousarray(np.asarray(W2, dtype=np.float32)),
        "b2": np.ascontiguousarray(np.asarray(b2, dtype=np.float32)),
        "w_score": np.ascontiguousarray(np.asarray(w_score, dtype=np.float32)),
    }
    in_maps = []
    for c in range(N_CORES):
        sl = slice(c * B, (c + 1) * B)
        in_maps.append({
            "query": query[sl],
            "keys": keys[sl],
            "values": values[sl],
            "mask": mask[sl],
            **shared,
        })
    return in_maps


def gather_out(results):
    out = np.concatenate([results[c]["out"] for c in range(N_CORES)], axis=0)
    return out.reshape(64, 1, D).astype(np.float32)


def kernel(query, keys, values, mask, W1, b1, W2, b2, w_score, b_score):
    """Full-input entry point: shards over 8 NeuronCores, returns [64, 1, D]."""
    from concourse.bass_utils import run_bass_kernel_spmd

    nc = _get_built()
    in_maps = make_in_maps(query, keys, values, mask, W1, b1, W2, b2, w_score)
    res = run_bass_kernel_spmd(nc, in_maps, core_ids=list(range(N_CORES)))
    return gather_out(res.results)



# revision 2
# speedup vs baseline: 2.2706x; 2.2706x over previous
"""DIN attention layer kernel for Trainium2 (8 NeuronCores, data-parallel).

Reference computation (per batch b):
    att = [q, k, q-k, q*k]            # [T, 4M]
    h1  = relu(att @ W1 + b1)         # [T, D]
    h2  = relu(h1 @ W2 + b2)          # [T, D]
    s   = h2 @ w_score + b_score      # [T, 1]
    attn = softmax(s.T + mask * -1e9) # [1, T]
    out = attn @ values               # [1, D]

Key optimizations (all exact math, bf16 matmuls / fp32 accumulation):
  * Mask gather on host: masked tokens have attn = exp(-1e9) = 0 exactly,
    so only the ~half unmasked tokens are shipped/computed. Batches are
    sorted by unmasked count and dealt into 8 "slots" (one batch per core
    per slot) so each slot's static token length is the max over just its
    8 batches (~52-55% of T).
  * Algebraic fold of the concat matmul, with q absorbed on the host:
        att @ W1 = [q@(W1a+W1c) + b1]  (per-batch bias row, host fp32)
                 + k @ ((W1b - W1c) + q∘W1d)   (per-batch weight, K=256)
    so mm1 contracts 256 instead of 1024.
  * All layout work on host: keys pre-transposed to [m, t], weights
    pre-combined and pre-cast to bf16, biases pre-striped. Device DMAs
    are plain [128, X] copies; no on-device transposes or weight casts.
  * Softmax without max-subtraction (scores are O(1); masked lanes are
    exp(-1e9) = 0), sum fused into the Exp activation via accum_out.
  * relu drains alternate between Scalar and Vector engines; attn@values
    for slot i is emitted inside slot i+1 (between mm1 and mm2) so the
    PE never waits on the softmax chain.
"""

import numpy as np
import ml_dtypes

P = 128
B_FULL = 64    # total batches
T = 1024       # tokens
M = 256        # key feature dim
D = 1024       # hidden dim
MC = M // P    # key-feature chunks (2)
DC = D // P    # hidden chunks (8)
N_CORES = 8
SLOTS = 8      # batches per core
NEG = -1.0e9
BF = ml_dtypes.bfloat16

_built_cache = {}


def _splits(t):
    if t <= 512:
        return [(0, t)]
    return [(0, 512), (512, t - 512)]


def _build(sizes):
    import concourse.bass as bass  # noqa: F401
    import concourse.bacc as bacc
    import concourse.mybir as mybir
    import concourse.tile as tile
    from contextlib import ExitStack

    F32 = mybir.dt.float32
    BF16 = mybir.dt.bfloat16
    AF = mybir.ActivationFunctionType
    OP = mybir.AluOpType

    tos = [(t + P - 1) // P for t in sizes]
    pis = [to * P for to in tos]
    T0, TO0, P0 = sizes[0], tos[0], pis[0]

    nc = bacc.Bacc("TRN2")
    kT_d = [nc.dram_tensor(f"kT{i}", [MC, P, sizes[i]], BF16, kind="ExternalInput").ap()
            for i in range(SLOTS)]
    w1e_d = [nc.dram_tensor(f"w1e{i}", [MC, P, D], BF16, kind="ExternalInput").ap()
             for i in range(SLOTS)]
    vals_d = [nc.dram_tensor(f"vals{i}", [pis[i], D], BF16, kind="ExternalInput").ap()
              for i in range(SLOTS)]
    rt_d = nc.dram_tensor("rt", [SLOTS, P, DC], F32, kind="ExternalInput").ap()
    mask_d = nc.dram_tensor("maskg", [SLOTS, P0], F32, kind="ExternalInput").ap()
    w2_d = nc.dram_tensor("W2", [DC, P, D], BF16, kind="ExternalInput").ap()
    b2_d = nc.dram_tensor("b2s", [P, DC], F32, kind="ExternalInput").ap()
    ws_d = nc.dram_tensor("wss", [P, DC], BF16, kind="ExternalInput").ap()
    out_d = nc.dram_tensor("out", [SLOTS, D], F32, kind="ExternalOutput").ap()

    with tile.TileContext(nc) as tc, ExitStack() as ctx:
        cons = ctx.enter_context(tc.tile_pool(name="cons", bufs=1))
        ktp = ctx.enter_context(tc.tile_pool(name="ktp", bufs=2))
        w1p = ctx.enter_context(tc.tile_pool(name="w1p", bufs=2))
        rtp = ctx.enter_context(tc.tile_pool(name="rtp", bufs=2))
        mkp = ctx.enter_context(tc.tile_pool(name="mkp", bufs=2))
        vpool = ctx.enter_context(tc.tile_pool(name="vp", bufs=2))
        h1pool = ctx.enter_context(tc.tile_pool(name="h1p", bufs=2))
        h2pool = ctx.enter_context(tc.tile_pool(name="h2p", bufs=2))
        small = ctx.enter_context(tc.tile_pool(name="small", bufs=2))
        dram = ctx.enter_context(tc.tile_pool(name="dram", bufs=2, space="DRAM"))
        psum_mm = ctx.enter_context(tc.tile_pool(name="psmm", bufs=4, space="PSUM"))
        psum_sc = ctx.enter_context(tc.tile_pool(name="pssc", bufs=2, space="PSUM"))
        psum_vec = ctx.enter_context(tc.tile_pool(name="psvec", bufs=2, space="PSUM"))

        # ---- one-time setup --------------------------------------------
        w2_sb = cons.tile([P, DC, D], BF16)
        for c in range(DC):
            nc.sync.dma_start(w2_sb[:, c, :], w2_d[c])
        b2_sb = cons.tile([P, DC], F32)
        nc.sync.dma_start(b2_sb, b2_d)
        ws_sb = cons.tile([P, DC], BF16)
        nc.sync.dma_start(ws_sb, ws_d)
        score_sb = cons.tile([1, P0], F32)
        nc.vector.memset(score_sb, NEG)
        exp_bf = cons.tile([1, P0], BF16)

        # ---- per-slot pipeline -----------------------------------------
        carry = {}

        def emit_attn_values(i):
            st = carry.pop(i)
            out_ps = [psum_vec.tile([1, 512], F32, tag="vec", name=f"o_ps{h}")
                      for h in range(2)]
            for h in range(2):
                for c2 in range(st["to"]):
                    nc.tensor.matmul(
                        out_ps[h],
                        st["attn_t"][:, c2:c2 + 1],
                        st["vals"][:, c2, h * 512:(h + 1) * 512],
                        start=(c2 == 0), stop=(c2 == st["to"] - 1),
                    )
            out_sb = small.tile([1, D], F32, tag="osb")
            for h in range(2):
                nc.vector.tensor_scalar_mul(
                    out_sb[:, h * 512:(h + 1) * 512], out_ps[h], st["rec"])
            nc.sync.dma_start(out_d[i:i + 1, :], out_sb)

        for i in range(SLOTS):
            ti, to_i, pi = sizes[i], tos[i], pis[i]
            sp = _splits(ti)

            kt = ktp.tile([P, MC, T0], BF16, tag="kt")
            for c in range(MC):
                nc.sync.dma_start(kt[:, c, :ti], kT_d[i][c])
            w1e = w1p.tile([P, MC, D], BF16, tag="w1e")
            for c in range(MC):
                nc.sync.dma_start(w1e[:, c, :], w1e_d[i][c])
            rt_t = rtp.tile([P, DC], F32, tag="rt")
            nc.sync.dma_start(rt_t, rt_d[i])
            mask_t = mkp.tile([1, P0], F32, tag="mask")
            nc.sync.dma_start(mask_t[:, :pi], mask_d[i:i + 1, :pi])
            vals_t = vpool.tile([P, TO0, D], BF16, tag="vals")
            for c2 in range(to_i):
                nc.sync.dma_start(vals_t[:, c2, :], vals_d[i][c2 * P:(c2 + 1) * P])

            # mm1: h1[d, t] = relu(w1e.T @ kT + rt)
            h1 = h1pool.tile([P, DC, T0], BF16, tag="H1")
            for j in range(DC):
                for (s0, sl) in sp:
                    ps = psum_mm.tile([P, 512], F32, tag="mm")
                    for c in range(MC):
                        nc.tensor.matmul(
                            ps[:, :sl], w1e[:, c, j * P:(j + 1) * P],
                            kt[:, c, s0:s0 + sl],
                            start=(c == 0), stop=(c == MC - 1),
                        )
                    if j % 2 == 0:
                        nc.scalar.activation(
                            h1[:, j, s0:s0 + sl], ps[:, :sl], AF.Relu,
                            bias=rt_t[:, j:j + 1], scale=1.0)
                    else:
                        nc.vector.tensor_scalar(
                            h1[:, j, s0:s0 + sl], ps[:, :sl],
                            rt_t[:, j:j + 1], 0.0, op0=OP.add, op1=OP.max)

            # deferred attn@values for the previous slot: the PE works on
            # it here while this slot's h1 relu chain drains
            if i > 0:
                emit_attn_values(i - 1)

            # mm2 + score: score for chunk j-1 interleaves with mm2 of j
            score_ps = [psum_sc.tile([1, 512], F32, tag="sc", name=f"s_ps{s}")
                        for s in range(len(sp))]
            h2_tiles = {}
            for j in range(DC):
                h2 = h2pool.tile([P, T0], BF16, tag="H2")
                h2_tiles[j] = h2
                for (s0, sl) in sp:
                    ps = psum_mm.tile([P, 512], F32, tag="mm")
                    for c in range(DC):
                        nc.tensor.matmul(
                            ps[:, :sl], w2_sb[:, c, j * P:(j + 1) * P],
                            h1[:, c, s0:s0 + sl],
                            start=(c == 0), stop=(c == DC - 1),
                        )
                    if j % 2 == 1:
                        nc.scalar.activation(
                            h2[:, s0:s0 + sl], ps[:, :sl], AF.Relu,
                            bias=b2_sb[:, j:j + 1], scale=1.0)
                    else:
                        nc.vector.tensor_scalar(
                            h2[:, s0:s0 + sl], ps[:, :sl],
                            b2_sb[:, j:j + 1], 0.0, op0=OP.add, op1=OP.max)
                if j > 0:
                    jj = j - 1
                    h2p = h2_tiles.pop(jj)
                    for si, (s0, sl) in enumerate(sp):
                        nc.tensor.matmul(
                            score_ps[si][:, :sl], ws_sb[:, jj:jj + 1],
                            h2p[:, s0:s0 + sl],
                            start=(jj == 0), stop=False, skip_group_check=True)
            jj = DC - 1
            h2p = h2_tiles.pop(jj)
            for si, (s0, sl) in enumerate(sp):
                nc.tensor.matmul(
                    score_ps[si][:, :sl], ws_sb[:, jj:jj + 1],
                    h2p[:, s0:s0 + sl],
                    start=False, stop=True, skip_group_check=True)

            # softmax: masked/padding lanes get -1e9 -> exp underflows to 0
            for si, (s0, sl) in enumerate(sp):
                nc.vector.scalar_tensor_tensor(
                    score_sb[:, s0:s0 + sl], in0=mask_t[:, s0:s0 + sl],
                    scalar=NEG, in1=score_ps[si][:, :sl],
                    op0=OP.mult, op1=OP.add)
            if ti < pi:
                nc.vector.memset(score_sb[:, ti:pi], NEG)
            sum_sb = small.tile([1, 1], F32, tag="sum")
            nc.scalar.activation(exp_bf[:, :pi], score_sb[:, :pi], AF.Exp,
                                 accum_out=sum_sb)
            rec = small.tile([1, 1], F32, tag="rec")
            nc.vector.reciprocal(rec, sum_sb)
            # attn_t[p, c] = exp_score[c*128 + p] via a DRAM bounce
            attn_dram = dram.tile([1, P0], BF16, tag="attn_dram")
            nc.sync.dma_start(attn_dram[:, :pi], exp_bf[:, :pi])
            attn_t = small.tile([P, TO0], BF16, tag="attn")
            nc.sync.dma_start(
                attn_t[:, :to_i],
                attn_dram[:, :pi].rearrange("one (c p) -> p (one c)", p=P))
            carry[i] = {"attn_t": attn_t, "vals": vals_t, "rec": rec, "to": to_i}

        emit_attn_values(SLOTS - 1)

    nc.compile()
    return nc


def _get_built(sizes):
    nc = _built_cache.get(sizes)
    if nc is None:
        nc = _build(sizes)
        _built_cache[sizes] = nc
    return nc


def prepare(query, keys, values, mask, W1, b1, W2, b2, w_score, b_score=None):
    """Host-side preprocessing: gather unmasked tokens, fold q into the
    layer-1 weights/bias, pre-stripe/pre-cast everything to device layout.
    Returns (sizes, in_maps, order)."""
    q = np.asarray(query, np.float32).reshape(B_FULL, M)
    keys = np.asarray(keys, np.float32).reshape(B_FULL, T, M)
    values = np.asarray(values, np.float32).reshape(B_FULL, T, D)
    mask = np.asarray(mask, np.float32).reshape(B_FULL, T)
    W1 = np.asarray(W1, np.float32)
    b1 = np.asarray(b1, np.float32)
    W2 = np.asarray(W2, np.float32)
    b2 = np.asarray(b2, np.float32)
    ws = np.asarray(w_score, np.float32).reshape(D)

    unm = mask == 0.0
    counts = unm.sum(1)
    order = np.argsort(-counts, kind="stable")
    sizes = []
    for i in range(SLOTS):
        mx = int(counts[order[i * N_CORES]])
        sizes.append(max(8, min(T, ((mx + 7) // 8) * 8)))
    sizes = tuple(sizes)
    tos = [(t + P - 1) // P for t in sizes]
    pis = [to * P for to in tos]
    P0 = pis[0]

    W1qc = W1[0:M] + W1[2 * M:3 * M]
    W1bc = W1[M:2 * M] - W1[2 * M:3 * M]
    W1d = W1[3 * M:4 * M]
    rt_all = (q @ W1qc + b1).astype(np.float32)           # [64, D]

    W2h = np.ascontiguousarray(W2.reshape(DC, P, D).astype(BF))
    b2s = np.ascontiguousarray(b2.reshape(DC, P).T)
    wss = np.ascontiguousarray(ws.reshape(DC, P).T.astype(BF))

    in_maps = []
    for c in range(N_CORES):
        im = {"W2": W2h, "b2s": b2s, "wss": wss}
        rt_core = np.zeros((SLOTS, P, DC), np.float32)
        maskg = np.ones((SLOTS, P0), np.float32)
        for i in range(SLOTS):
            b = int(order[i * N_CORES + c])
            ti, pi = sizes[i], pis[i]
            idx = np.nonzero(unm[b])[0]
            n = len(idx)
            kt = np.zeros((MC, P, ti), BF)
            kt[:, :, :n] = keys[b, idx].T.reshape(MC, P, n).astype(BF)
            im[f"kT{i}"] = kt
            im[f"w1e{i}"] = np.ascontiguousarray(
                (W1bc + q[b][:, None] * W1d).reshape(MC, P, D).astype(BF))
            va = np.zeros((pi, D), BF)
            va[:n] = values[b, idx].astype(BF)
            im[f"vals{i}"] = va
            rt_core[i] = rt_all[b].reshape(DC, P).T
            maskg[i, :n] = 0.0
        im["rt"] = rt_core
        im["maskg"] = maskg
        in_maps.append(im)
    return sizes, in_maps, order


def gather_out(results, order):
    out = np.zeros((B_FULL, D), np.float32)
    for c in range(N_CORES):
        o = np.asarray(results[c]["out"], np.float32)
        for i in range(SLOTS):
            out[order[i * N_CORES + c]] = o[i]
    return out.reshape(B_FULL, 1, D)


def kernel(query, keys, values, mask, W1, b1, W2, b2, w_score, b_score):
    """Full-input entry point: shards over 8 NeuronCores, returns [64, 1, D]."""
    from concourse.bass_utils import run_bass_kernel_spmd

    sizes, in_maps, order = prepare(query, keys, values, mask,
                                    W1, b1, W2, b2, w_score)
    nc = _get_built(sizes)
    res = run_bass_kernel_spmd(nc, in_maps, core_ids=list(range(N_CORES)))
    return gather_out(res.results, order)
